# revision 2
# baseline (speedup 1.0000x reference)
"""Trainium2 Bass kernel for DeepGCN (nn_DeepGCN_82454782148693) — v2.

Changes vs v1 baseline:
  - bf16 everywhere on the hot path (x, h', messages, S, weights, h2);
    fp32 PSUM accumulation and fp32 BN statistics.
  - Feature-major aggregation: matmul(lhsT=msgs [e,f], rhs=S [e,d]) gives
    agg [f, d] directly; finalize is one scalar_tensor_tensor multiply by a
    broadcast dinv row (applies dinv_dst) with no per-tile transposes.
  - Self-loops handled by pre-initializing the PSUM accumulation banks with
    h'T (which already carries one dinv factor) via a DVE copy.
  - One-hot S generated 8 chunks at a time with a single tensor_tensor
    is_equal against a stride-0 broadcast view of the dst-slot row.
  - dma_gather in prepare_only mode + trigger_dma so descriptor prep
    overlaps the DMA transfers; edges sorted by src within each chunk run
    for HBM locality.
  - conv bias folded into lin bias on the host.
"""

import os
import sys

import numpy as np

for _p in ("/opt/trn_rl_repo", "/root/.axon_site/_ro/trn_rl_repo"):
    if os.path.isdir(_p) and _p not in sys.path:
        sys.path.append(_p)

import ml_dtypes

import concourse.bass as bass
import concourse.bacc as bacc
import concourse.mybir as mybir
import concourse.tile as tile
from concourse import bass_utils

F32 = mybir.dt.float32
BF16 = mybir.dt.float16
I16 = mybir.dt.int16
AF = mybir.ActivationFunctionType
OP = mybir.AluOpType
AX = mybir.AxisListType
BF16NP = np.float16

H = 128


class Cfg:
    def __init__(self):
        self.N, self.E, self.NCORES = 100000, 1600000, 8
        self.H, self.L, self.HC, self.C = 128, 4, 64, 2
        self.ALPHA, self.THETA, self.EPS = 0.1, 0.5, 1e-5
        self.NSH = self.N // self.NCORES          # 12500
        self.TILES = (self.NSH + 127) // 128      # 98
        self.NPAD = self.TILES * 128              # 12544
        self.NP = self.NCORES * self.NPAD         # 100352
        self.BANKS = 4
        self.BROWS = self.NP // self.BANKS        # 25088
        assert self.BROWS <= 32768
        self.GT = 7
        self.NG = self.TILES // self.GT           # 14
        self.GMAX = 8                             # max chunks per gather call
        # node chunks of 512 for dense phases
        self.PCH = []
        off = 0
        while off < self.NPAD:
            w = min(512, self.NPAD - off)
            self.PCH.append((off, w))
            off += w
        self.LPARTS = [(0, 512), (512, 384)]      # parts covering GT*128


CFG = Cfg()


class Sched:
    """Static gather/aggregation schedule, shared by all cores."""

    def __init__(self, CH_tb):
        c = CFG
        self.CH_tb = CH_tb                        # [TILES, BANKS] chunks
        self.chunk_off = {}                       # (t, b) -> chunk index base
        off = 0
        self.group_base = []                      # chunk base per group
        self.gb_range = {}                        # (g, b) -> (chunk0, nchunks)
        for g in range(c.NG):
            self.group_base.append(off)
            for b in range(c.BANKS):
                b0 = off
                for tp in range(c.GT):
                    t = g * c.GT + tp
                    self.chunk_off[(t, b)] = off
                    off += int(CH_tb[t, b])
                self.gb_range[(g, b)] = (b0, off - b0)
        self.NCHUNK = off
        self.TOTSLOTS = off * 128


def build_schedule(edge_index, cfg):
    c = cfg
    src = np.asarray(edge_index[0], np.int64)
    dst = np.asarray(edge_index[1], np.int64)

    deg = np.bincount(dst, minlength=c.N).astype(np.float32) + 1.0
    dinv = 1.0 / np.sqrt(np.maximum(deg, 1.0))

    core = dst // c.NSH
    ldst = dst - core * c.NSH
    tile_id = ldst >> 7
    dloc = ldst & 127
    spad = (src // c.NSH) * c.NPAD + (src % c.NSH)
    bank = spad // c.BROWS
    bidx = (spad - bank * c.BROWS).astype(np.int64)

    counts = np.zeros((c.NCORES, c.TILES, c.BANKS), dtype=np.int64)
    np.add.at(counts, (core, tile_id, bank), 1)
    CH_tb = np.ceil(counts.max(axis=0) / 128.0).astype(np.int64)  # [TILES,BANKS]
    CH_tb = np.maximum(CH_tb, 1)
    sched = Sched(CH_tb)

    # place edges: sort by (core, tile, bank, src) for locality
    order = np.lexsort((src, bank, tile_id, core))
    c_s, t_s, b_s = core[order], tile_id[order], bank[order]
    bi_s, dl_s = bidx[order], dloc[order]
    key = (c_s * c.TILES + t_s) * c.BANKS + b_s
    runstart = np.r_[0, np.flatnonzero(np.diff(key)) + 1]
    runid = np.zeros(len(key), dtype=np.int64)
    runid[runstart[1:]] = 1
    runid = np.cumsum(runid)
    pos = np.arange(len(key)) - runstart[runid]

    base = np.array([sched.chunk_off[(t, b)] * 128
                     for t in range(c.TILES) for b in range(c.BANKS)],
                    dtype=np.int64).reshape(c.TILES, c.BANKS)
    slots = base[t_s, b_s] + pos

    idx_slots = np.zeros((c.NCORES, sched.TOTSLOTS), dtype=np.int16)
    dst_slots = np.full((c.NCORES, sched.TOTSLOTS), 300.0, dtype=np.float32)
    idx_slots[c_s, slots] = bi_s.astype(np.int16)
    dst_slots[c_s, slots] = dl_s.astype(np.float32)

    # wrapped idx [NCORES, 16, TOT/16] replicated to 128 partitions
    idx_wrapped = idx_slots.reshape(c.NCORES, sched.TOTSLOTS // 16, 16)
    idx_wrapped = idx_wrapped.transpose(0, 2, 1)
    idx_in = np.tile(idx_wrapped, (1, 8, 1)).astype(np.int16)

    dst_in = dst_slots.reshape(c.NCORES, sched.NCHUNK, 128)
    dst_in = dst_in.transpose(0, 2, 1).astype(BF16NP)     # [NCORES,128,NCHUNK]

    # dinv broadcast row per core: [128, NPAD] bf16
    dpad = np.ones((c.NCORES, c.NPAD), np.float32)
    dpad[:, :c.NSH] = dinv.reshape(c.NCORES, c.NSH)
    dinv_in = np.broadcast_to(dpad[:, None, :], (c.NCORES, 128, c.NPAD))
    dinv_in = np.ascontiguousarray(dinv_in).astype(BF16NP)

    return sched, idx_in, dst_in, dinv_in


def pack_weights(inputs, cfg):
    c = cfg
    L, HC, Cc = c.L, c.HC, c.C
    cols = [np.asarray(inputs["proj_W"], np.float32)]
    for l in range(L):
        cols.append(np.asarray(inputs["conv_W"][l], np.float32))
    for l in range(L):
        cols.append(np.asarray(inputs["lin_W"][l], np.float32))
    cols.append(np.asarray(inputs["cls_W1"], np.float32))          # [128,64]
    w2 = np.zeros((H, Cc), np.float32)
    w2[:HC] = np.asarray(inputs["cls_W2"], np.float32)
    cols.append(w2)
    W = np.concatenate(cols, axis=1).astype(BF16NP)

    # biases fp32: proj_b | lin_b_eff x4 | bn_g x4 | bn_b x4 | cls_b1 | cls_b2
    nb = np.zeros((H, 15), np.float32)
    nb[:, 0] = np.asarray(inputs["proj_b"], np.float32)
    for l in range(L):
        lin_b_eff = (np.asarray(inputs["conv_b"][l], np.float64)
                     @ np.asarray(inputs["lin_W"][l], np.float64)
                     + np.asarray(inputs["lin_b"][l], np.float64))
        nb[:, 1 + l] = lin_b_eff.astype(np.float32)
        nb[:, 5 + l] = np.asarray(inputs["bn_g"][l], np.float32)
        nb[:, 9 + l] = np.asarray(inputs["bn_b"][l], np.float32)
    nb[:HC, 13] = np.asarray(inputs["cls_b1"], np.float32)
    nb[:Cc, 14] = np.asarray(inputs["cls_b2"], np.float32)
    return W, nb


# ----------------------------------------------------------------------------
# Device program
# ----------------------------------------------------------------------------

def build_program(cfg, sched):
    c = cfg
    L = c.L
    WCOLS = 128 * (1 + 2 * L) + c.HC + c.C
    C1 = float(1.0 - c.ALPHA - c.THETA)
    NCHUNK = sched.NCHUNK
    IDXCOLS = sched.TOTSLOTS // 16

    nc = bacc.Bacc("TRN2", target_bir_lowering=False, debug=False,
                   enable_asserts=False, num_devices=c.NCORES)

    # ---- I/O ----
    xT_in = nc.dram_tensor("xT_in", [H, c.NPAD], BF16, kind="ExternalInput").ap()
    dinv_in = nc.dram_tensor("dinv_in", [H, c.NPAD], BF16,
                             kind="ExternalInput").ap()
    idx_in = nc.dram_tensor("idx_in", [H, IDXCOLS], I16,
                            kind="ExternalInput").ap()
    dst_in = nc.dram_tensor("dst_in", [H, NCHUNK], BF16,
                            kind="ExternalInput").ap()
    w_in = nc.dram_tensor("w_in", [H, WCOLS], BF16, kind="ExternalInput").ap()
    b_in = nc.dram_tensor("b_in", [H, 15], F32, kind="ExternalInput").ap()
    out_d = nc.dram_tensor("out_d", [c.C, c.NPAD], F32,
                           kind="ExternalOutput").ap()
    DEBUG = bool(int(os.environ.get("GCN_DEBUG", "0")))
    if DEBUG:
        dbgx_d = nc.dram_tensor("dbgx_d", [H, c.NPAD], BF16,
                                kind="ExternalOutput").ap()
        dbgh_d = nc.dram_tensor("dbgh_d", [H, c.NPAD], BF16,
                                kind="ExternalOutput").ap()
        dbgt_d = nc.dram_tensor("dbgt_d", [H, c.NPAD], BF16,
                                kind="ExternalOutput").ap()

    # ---- internal DRAM ----
    hsh_d = nc.dram_tensor("hsh_d", [c.NPAD, H], BF16, kind="Internal").ap()
    hall_d = nc.dram_tensor("hall_d", [c.NP, H], BF16, kind="Internal",
                            addr_space="Shared").ap()
    h2_d = nc.dram_tensor("h2_d", [H, c.NPAD], BF16, kind="Internal").ap()
    stin_d = nc.dram_tensor("stin_d", [H, 2], F32, kind="Internal").ap()
    stout_d = nc.dram_tensor("stout_d", [8 * H, 2], F32, kind="Internal",
                             addr_space="Shared").ap()

    # ---- SBUF residents ----
    xT = nc.alloc_sbuf_tensor("xT", [H, c.NPAD], BF16).ap()
    x0 = nc.alloc_sbuf_tensor("x0", [H, c.NPAD], BF16).ap()
    hT = nc.alloc_sbuf_tensor("hT", [H, c.NPAD], BF16).ap()
    hTd = nc.alloc_sbuf_tensor("hTd", [H, c.NPAD], BF16).ap()
    dinvb = nc.alloc_sbuf_tensor("dinvb", [H, c.NPAD], BF16).ap()
    dstb = nc.alloc_sbuf_tensor("dstb", [H, NCHUNK], BF16).ap()
    iotb = nc.alloc_sbuf_tensor("iotb", [H, H], BF16).ap()
    zeros = nc.alloc_sbuf_tensor("zeros", [H, 512], BF16).ap()
    ident = nc.alloc_sbuf_tensor("ident", [H, H], BF16).ap()
    wsb = nc.alloc_sbuf_tensor("wsb", [H, WCOLS], BF16).ap()
    bsb = nc.alloc_sbuf_tensor("bsb", [H, 15], F32).ap()
    sums = nc.alloc_sbuf_tensor("sums", [H, 32], F32).ap()
    sqs = nc.alloc_sbuf_tensor("sqs", [H, 32], F32).ap()
    stat = nc.alloc_sbuf_tensor("stat", [H, 12], F32).ap()

    wproj = wsb[:, 0:128]
    wconv = lambda l: wsb[:, 128 * (1 + l):128 * (2 + l)]
    wlin = lambda l: wsb[:, 128 * (1 + L + l):128 * (2 + L + l)]
    wcls1 = wsb[:, 128 * (1 + 2 * L):128 * (1 + 2 * L) + c.HC]
    wcls2 = wsb[:c.HC, 128 * (1 + 2 * L) + c.HC:WCOLS]

    rg = [list(range(c.NCORES))]
    hall_banks = [hall_d[b * c.BROWS:(b + 1) * c.BROWS, :]
                  for b in range(c.BANKS)]

    with tile.TileContext(nc) as tc:
        # ================= P0: prologue =================
        with tc.sbuf_pool(name="p0", bufs=3) as pool, \
             tc.psum_pool(name="p0p", bufs=2) as pp:
            nc.sync.dma_start(wsb, w_in)
            nc.sync.dma_start(bsb, b_in)
            nc.sync.dma_start(dstb, dst_in)
            nc.sync.dma_start(dinvb, dinv_in)
            nc.gpsimd.iota(iotb, pattern=[[1, H]], base=0, channel_multiplier=0,
                           allow_small_or_imprecise_dtypes=True)
            pidx = pool.tile([H, H], BF16)
            nc.gpsimd.iota(pidx, pattern=[[0, H]], base=0, channel_multiplier=1,
                           allow_small_or_imprecise_dtypes=True)
            nc.vector.tensor_tensor(ident, iotb, pidx, OP.is_equal)
            nc.vector.memset(zeros, 0.0)
            for (off, w) in c.PCH:
                xin = pool.tile([H, 512], BF16, tag="xin")
                nc.sync.dma_start(xin[:, :w], xT_in[:, off:off + w])
                ps = pp.tile([H, 512], F32, tag="ps")
                nc.tensor.matmul(ps[:, :w], wproj, xin[:, :w])
                nc.scalar.activation(xT[:, off:off + w], ps[:, :w], AF.Relu,
                                     bias=bsb[:, 0:1], scale=1.0)
                nc.scalar.activation(x0[:, off:off + w], ps[:, :w], AF.Relu,
                                     bias=bsb[:, 0:1], scale=1.0)
            if c.NSH < c.NPAD:
                nc.vector.memset(xT[:, c.NSH:c.NPAD], 0.0)
                nc.vector.memset(x0[:, c.NSH:c.NPAD], 0.0)

        # ================= layers =================
        for li in range(L):
            # ---- P1: h'T = (x @ convW) * dinv (resident) ; node-major DRAM
            with tc.sbuf_pool(name=f"l{li}a", bufs=3) as pool, \
                 tc.psum_pool(name=f"l{li}ap", bufs=2) as pp, \
                 tc.psum_pool(name=f"l{li}at", bufs=2) as pt:
                for (off, w) in c.PCH:
                    ps = pp.tile([H, 512], F32, tag="ps")
                    nc.tensor.matmul(ps[:, :w], wconv(li), xT[:, off:off + w])
                    nc.vector.scalar_tensor_tensor(
                        hT[:, off:off + w], ps[:, :w], 1.0,
                        dinvb[:, off:off + w], op0=OP.mult, op1=OP.mult)
                    nc.vector.tensor_tensor(
                        hTd[:, off:off + w], hT[:, off:off + w],
                        dinvb[:, off:off + w], OP.mult)
                    tp2 = pt.tile([H, 512], BF16, tag="tp2")
                    for j in range(w // 128):
                        nc.tensor.transpose(
                            tp2[:, j * 128:(j + 1) * 128],
                            hT[:, off + j * 128:off + (j + 1) * 128], ident)
                    stg = pool.tile([H, 512], BF16, tag="stg")
                    nc.scalar.activation(stg[:, :w], tp2[:, :w], AF.Identity,
                                         bias=0.0, scale=1.0)
                    dram = hsh_d[off:off + w, :].rearrange(
                        "(j p) f -> p j f", p=128)
                    nc.sync.dma_start(
                        dram, stg[:, :w].rearrange("p (j f) -> p j f", f=H))

            # ---- P2: AllGather h' ----
            nc.gpsimd.collective_compute(
                "AllGather", OP.bypass, replica_groups=rg,
                ins=[hsh_d], outs=[hall_d])

            # ---- P3: gather + one-hot segment-sum + lin + stats ----
            GBMAX = max(sched.gb_range[(g, b)][1]
                        for g in range(c.NG) for b in range(c.BANKS))
            with tc.sbuf_pool(name=f"l{li}g", bufs=3) as pool, \
                 tc.sbuf_pool(name=f"l{li}s", bufs=6) as spool, \
                 tc.psum_pool(name=f"l{li}ga", bufs=2) as ppa, \
                 tc.psum_pool(name=f"l{li}gl", bufs=2) as ppl:
                sc = 0
                for g in range(c.NG):
                    gb = c.GT * 128 * g                    # node col base
                    cb = sched.group_base[g]               # chunk base

                    aggA = ppa.tile([H, 512], F32, tag="aggA", name="aggA")
                    aggB = ppa.tile([H, 384], F32, tag="aggB", name="aggB")
                    nc.tensor.matmul(aggA, ident, zeros[:, :512], start=True,
                                     stop=False, skip_group_check=True)
                    nc.tensor.matmul(aggB, ident, zeros[:, :384], start=True,
                                     stop=False, skip_group_check=True)

                    def agg_slice(tp):
                        if tp < 4:
                            return aggA[:, tp * 128:(tp + 1) * 128]
                        return aggB[:, (tp - 4) * 128:(tp - 3) * 128]

                    # last matmul per psum region (bank-major emission order)
                    last_for = {}
                    for b in range(c.BANKS):
                        for tp in range(c.GT):
                            t = g * c.GT + tp
                            for j in range(int(sched.CH_tb[t, b])):
                                last_for[0 if tp < 4 else 1] = (b, tp, j)

                    for b in range(c.BANKS):
                        b0, nch = sched.gb_range[(g, b)]
                        idxt = pool.tile([H, GBMAX * 8], I16, tag="idxt")
                        nc.sync.dma_start(idxt[:, :nch * 8],
                                          idx_in[:, b0 * 8:(b0 + nch) * 8])
                        msgs = pool.tile([H, GBMAX * 128], BF16, tag="msgs")
                        j0 = 0
                        while j0 < nch:
                            jn = min(c.GMAX, nch - j0)
                            nc.gpsimd.dma_gather(
                                out_ap=msgs[:, j0 * 128:(j0 + jn) * 128]
                                .rearrange("p (ch f) -> p ch f", f=H),
                                in_ap=hall_banks[b],
                                idxs_ap=idxt[:, j0 * 8:(j0 + jn) * 8],
                                num_idxs=jn * 128,
                                num_idxs_reg=jn * 128,
                                elem_size=H,
                            )
                            j0 += jn
                        # S generation, 8 chunks per op
                        Stiles = []
                        for s0 in range(0, nch, 8):
                            sn = min(8, nch - s0)
                            St = spool.tile([H, 1024], BF16, tag="St")
                            in0 = iotb.unsqueeze(1).broadcast_to([H, sn, H])
                            in1 = dstb[:, b0 + s0:b0 + s0 + sn].unsqueeze(
                                2).broadcast_to([H, sn, H])
                            nc.vector.tensor_tensor(
                                St[:, :sn * 128].rearrange(
                                    "p (c f) -> p c f", f=H),
                                in0, in1, OP.is_equal)
                            Stiles.append(St)
                        # matmuls for this bank's chunks, tile by tile
                        for tp in range(c.GT):
                            t = g * c.GT + tp
                            n = int(sched.CH_tb[t, b])
                            cj0 = sched.chunk_off[(t, b)] - b0
                            for j in range(n):
                                cj = cj0 + j
                                region = 0 if tp < 4 else 1
                                is_last = last_for[region] == (b, tp, j)
                                nc.tensor.matmul(
                                    agg_slice(tp),
                                    msgs[:, cj * 128:(cj + 1) * 128],
                                    Stiles[cj // 8][:, (cj % 8) * 128:
                                                    (cj % 8 + 1) * 128],
                                    start=False, stop=is_last,
                                    skip_group_check=True)

                    # finalize: yT = agg * dinv_dst   (feature-major, bf16)
                    yT = pool.tile([H, 896], BF16, tag="yT")
                    nc.vector.scalar_tensor_tensor(
                        yT[:, :512], aggA, 1.0, dinvb[:, gb:gb + 512],
                        op0=OP.mult, op1=OP.mult)
                    nc.vector.scalar_tensor_tensor(
                        yT[:, 512:], aggB, 1.0, dinvb[:, gb + 512:gb + 896],
                        op0=OP.mult, op1=OP.mult)

                    # lin + bias + stats, spill h2
                    for (off, w) in c.LPARTS:
                        ps3 = ppl.tile([H, 512], F32, tag="ps3")
                        nc.tensor.matmul(ps3[:, :w], wlin(li),
                                         yT[:, off:off + w],
                                         start=True, stop=False,
                                         skip_group_check=True)
                        nc.tensor.matmul(ps3[:, :w], wlin(li),
                                         hTd[:, gb + off:gb + off + w],
                                         start=False, stop=True,
                                         skip_group_check=True)
                        h2t = pool.tile([H, 512], BF16, tag="h2t")
                        gcol = gb + off
                        is_pad = gcol + w > c.NSH
                        if not is_pad:
                            nc.vector.tensor_scalar(
                                h2t[:, :w], ps3[:, :w], bsb[:, 1 + li:2 + li],
                                None, op0=OP.add, op1=OP.add,
                                accum_out=sums[:, sc:sc + 1])
                        else:
                            nc.vector.tensor_scalar(
                                h2t[:, :w], ps3[:, :w], bsb[:, 1 + li:2 + li],
                                None, op0=OP.add)
                            nc.vector.memset(h2t[:, c.NSH - gcol:w], 0.0)
                            nc.vector.tensor_reduce(
                                sums[:, sc:sc + 1], h2t[:, :w], AX.X, OP.add)
                        sq = pool.tile([H, 512], F32, tag="sq")
                        nc.vector.scalar_tensor_tensor(
                            sq[:, :w], h2t[:, :w], 0.0, h2t[:, :w],
                            op0=OP.add, op1=OP.mult,
                            accum_out=sqs[:, sc:sc + 1])
                        sc += 1
                        nc.sync.dma_start(h2_d[:, gcol:gcol + w], h2t[:, :w])
                nparts = sc

            # ---- P4: stats allreduce + scale/shift ----
            with tc.sbuf_pool(name=f"l{li}r", bufs=2) as pool:
                nc.vector.tensor_reduce(stat[:, 0:1], sums[:, :nparts], AX.X,
                                        OP.add)
                nc.vector.tensor_reduce(stat[:, 1:2], sqs[:, :nparts], AX.X,
                                        OP.add)
                nc.sync.dma_start(stin_d, stat[:, 0:2])
                nc.gpsimd.collective_compute(
                    "AllGather", OP.bypass, replica_groups=rg,
                    ins=[stin_d], outs=[stout_d])
                stall = pool.tile([H, 16], F32, tag="stall")
                nc.sync.dma_start(
                    stall.rearrange("p (k t) -> p k t", t=2),
                    stout_d.rearrange("(k p) t -> p k t", p=H))
                nc.vector.tensor_reduce(
                    stat[:, 2:3], stall.rearrange("p (k t) -> p k t", t=2)
                    [:, :, 0], AX.X, OP.add)
                nc.vector.tensor_reduce(
                    stat[:, 3:4], stall.rearrange("p (k t) -> p k t", t=2)
                    [:, :, 1], AX.X, OP.add)
                invn = 1.0 / float(c.N)
                nc.vector.tensor_scalar_mul(stat[:, 4:5], stat[:, 2:3], invn)
                m2 = pool.tile([H, 1], F32)
                nc.vector.tensor_tensor(m2, stat[:, 4:5], stat[:, 4:5], OP.mult)
                nc.vector.scalar_tensor_tensor(stat[:, 5:6], stat[:, 3:4], invn,
                                               m2, op0=OP.mult, op1=OP.subtract)
                vps = pool.tile([H, 1], F32)
                nc.vector.tensor_scalar_add(vps, stat[:, 5:6], float(c.EPS))
                sd = pool.tile([H, 1], F32)
                nc.scalar.sqrt(sd, vps)
                inv = pool.tile([H, 1], F32)
                nc.vector.reciprocal(inv, sd)
                gi = pool.tile([H, 1], F32)
                nc.vector.tensor_tensor(gi, inv, bsb[:, 5 + li:6 + li], OP.mult)
                nc.vector.tensor_scalar_mul(stat[:, 6:7], gi, C1)
                ms = pool.tile([H, 1], F32)
                nc.vector.tensor_tensor(ms, stat[:, 4:5], stat[:, 6:7], OP.mult)
                nc.vector.scalar_tensor_tensor(stat[:, 7:8],
                                               bsb[:, 9 + li:10 + li],
                                               C1, ms, op0=OP.mult,
                                               op1=OP.subtract)

            # ---- P5: x = relu(s*h2 + u + alpha*x0 + theta*x_prev) ----
            with tc.sbuf_pool(name=f"l{li}f", bufs=3) as pool:
                for (off, w) in c.PCH:
                    h2c = pool.tile([H, 512], BF16, tag="h2c")
                    nc.sync.dma_start(h2c[:, :w], h2_d[:, off:off + w])
                    t1 = pool.tile([H, 512], BF16, tag="t1")
                    nc.scalar.activation(t1[:, :w], h2c[:, :w], AF.Identity,
                                         bias=stat[:, 7:8], scale=stat[:, 6:7])
                    t2 = pool.tile([H, 512], BF16, tag="t2")
                    nc.vector.scalar_tensor_tensor(
                        t2[:, :w], xT[:, off:off + w], float(c.THETA),
                        t1[:, :w], op0=OP.mult, op1=OP.add)
                    t3 = pool.tile([H, 512], BF16, tag="t3")
                    nc.vector.scalar_tensor_tensor(
                        t3[:, :w], x0[:, off:off + w], float(c.ALPHA),
                        t2[:, :w], op0=OP.mult, op1=OP.add)
                    nc.scalar.activation(xT[:, off:off + w], t3[:, :w], AF.Relu,
                                         bias=0.0, scale=1.0)

            if DEBUG and li == 0:
                with tc.sbuf_pool(name="dbg", bufs=2) as pool:
                    nc.sync.dma_start(dbgx_d, xT)
                    nc.sync.dma_start(dbgt_d, hT)
                    for (off, w) in c.PCH:
                        t = pool.tile([H, 512], BF16, tag="db")
                        nc.sync.dma_start(t[:, :w], h2_d[:, off:off + w])
                        nc.sync.dma_start(dbgh_d[:, off:off + w], t[:, :w])

        # ================= P6: classifier =================
        with tc.sbuf_pool(name="p6", bufs=3) as pool, \
             tc.psum_pool(name="p6p", bufs=2) as pp, \
             tc.psum_pool(name="p6q", bufs=2) as pq:
            for (off, w) in c.PCH:
                ps = pp.tile([c.HC, 512], F32, tag="ps")
                nc.tensor.matmul(ps[:, :w], wcls1, xT[:, off:off + w])
                h3 = pool.tile([c.HC, 512], BF16, tag="h3")
                nc.scalar.activation(h3[:, :w], ps[:, :w], AF.Relu,
                                     bias=bsb[:c.HC, 13:14], scale=1.0)
                ps2 = pq.tile([c.C, 512], F32, tag="ps2")
                nc.tensor.matmul(ps2[:, :w], wcls2, h3[:, :w])
                ot = pool.tile([c.C, 512], F32, tag="ot")
                nc.vector.tensor_scalar(ot[:, :w], ps2[:, :w],
                                        bsb[:c.C, 14:15], None, op0=OP.add)
                nc.sync.dma_start(out_d[:, off:off + w], ot[:, :w])

    nc.compile()
    return nc


# ----------------------------------------------------------------------------
# Full pipeline
# ----------------------------------------------------------------------------

LAST_RESULTS = None
_PROGRAM_CACHE = {}


def kernel(**inputs):
    global LAST_RESULTS
    c = CFG
    x = np.ascontiguousarray(np.asarray(inputs["x"], np.float32))
    edge_index = np.asarray(inputs["edge_index"])
    sched, idx_in, dst_in, dinv_in = build_schedule(edge_index, c)
    W, B = pack_weights(inputs, c)

    key = (tuple(sched.CH_tb.reshape(-1).tolist()), os.environ.get('GCN_DEBUG', '0'))
    if key not in _PROGRAM_CACHE:
        _PROGRAM_CACHE[key] = build_program(c, sched)
    nc = _PROGRAM_CACHE[key]

    in_maps = []
    for k in range(c.NCORES):
        xs = np.zeros((H, c.NPAD), BF16NP)
        xs[:, :c.NSH] = x[k * c.NSH:(k + 1) * c.NSH].T.astype(BF16NP)
        in_maps.append({
            "xT_in": xs,
            "dinv_in": np.ascontiguousarray(dinv_in[k]),
            "idx_in": np.ascontiguousarray(idx_in[k]),
            "dst_in": np.ascontiguousarray(dst_in[k]),
            "w_in": W,
            "b_in": B,
        })

    trace = bool(int(os.environ.get("GCN_TRACE", "0")))
    res = bass_utils.run_bass_kernel_spmd(
        nc, in_maps, core_ids=list(range(c.NCORES)), trace=trace)
    LAST_RESULTS = res

    out = np.empty((c.N, c.C), np.float32)
    for k in range(c.NCORES):
        o = res.results[k]["out_d"]
        out[k * c.NSH:(k + 1) * c.NSH] = o[:, :c.NSH].T
    return out
